# revision 6
# baseline (speedup 1.0000x reference)
"""Trainium2 Bass kernel for CPELayer_ResAG (concept-routed LoRA edit layer).

Computation (per token t with concept c = concept_idx[t]):
    down = edit_direction[t] @ lora_down[c]          # [768]@[768,4] -> [4]
    up   = down @ lora_up[c]                         # [4]@[4,1280]  -> [1280]
    out  = x[t] @ W.T + b_lin + 0.25 * up

Strategy: data-parallel over batch across 8 cores (616 tokens/core, padded
to 640 = 5 x 128 token tiles).  The routed LoRA runs densely over all 50
concepts: A.T[(c,r), t] = lora_down_flat.T @ ed.T, masked on-device with a
one-hot built from is_equal against the broadcast token->concept ids (the
MoE routing), then contracted with lora_up_flat on the tensor engine into
the same PSUM as the org matmul.  The bias rides along as one extra
contraction row (ones row in the masked operand, b_lin row in luB).

Perf notes (vs the fp32r v1 at 51.3us):
 - org operands, lora_up and the output are bf16: halves the HBM traffic
   that saturated the 16 DMA queues (bf16 matmul streams at the same
   1 cycle/row as fp32r, so PE time is unchanged).
 - ed / lora_down are fp8e4 (lora_down pre-scaled x64 host-side; the
   one-hot mask carries 1/64 to undo it).  The LoRA branch is ~0.7% of
   the output scale, so fp8 rounding there is invisible.
 - inputs load as few large DMAs split across BOTH HWDGE rings (sync +
   scalar sequencers) -- v1 serialized 23 issues x ~0.7us on sync alone.
 - the token->concept ids broadcast across partitions via a K=1 matmul
   (ones.T @ idx) instead of a 128-descriptor broadcast DMA.
 - dummy warmup matmuls run during the load phase so the HAM clock gate
   reaches 2.4 GHz before the real matmuls start.

All host-side work is layout only (pad / transpose / reshape / dtype
casts of constants); every FLOP of the reference runs on device.
"""

import sys
import types

import numpy as np

import concourse.mybir as mybir
import concourse.tile as tile
from concourse import bacc
from concourse.bass_utils import run_bass_kernel_spmd

# If BASS_TRACE is set in the environment, run_bass_kernel_spmd imports
# antenv.axon_hooks, which some containers lack; stub it (None hook ->
# tracing is skipped gracefully, execution unaffected).
try:
    import antenv.axon_hooks  # noqa: F401
except ImportError:
    _m = types.ModuleType("antenv.axon_hooks")
    _m.get_axon_ntff_profile_hook = lambda: None
    _m.set_axon_ntff_profile_hook = lambda h: None
    sys.modules["antenv.axon_hooks"] = _m

# Problem shapes (hardcoded per spec nn_CPELayer_ResAG_19335942766951)
N_CORES = 8
B, T, DIN, DOUT = 64, 77, 768, 1280
N_CONCEPTS, RANK = 50, 4
SCALE = 0.25                # alpha/rank = 1/4, exact power of two
BPC = B // N_CORES          # batches per core = 8
TOK = BPC * T               # real tokens per core = 616
TOKP = 640                  # padded tokens = 5 full 128-token tiles
NT = TOKP // 128            # 5 token tiles
NJ = N_CONCEPTS * RANK      # 200 flattened (concept, rank) rows
KJ_PAD = 256                # padded rows: 200 lora + 1 bias + 55 zero
P = 128
KD = DIN // P               # 6 k-tiles of the d_in contraction
KA = 3                      # org k-tiles in wave A (k0..2), wave B k3..5
NH = TOKP // 2              # 320-token halves for the A.T psum tiles
LD_SCALE = 64.0             # fp8 pre-scale on lora_down; mask carries 1/64
N_CHUNKS = [(0, 512), (512, 512), (1024, 256)]
IDX_PAD = -2.0              # pad-token id; cvals pad is -1 so never equal

_cache = {}


def _build_bass():
    nc = bacc.Bacc("TRN2", target_bir_lowering=False, debug=False,
                   num_devices=N_CORES)
    f32 = mybir.dt.float32
    bf = mybir.dt.bfloat16
    f8 = mybir.dt.float8e4

    xT_d = nc.dram_tensor("xT", [DIN, TOKP], bf, kind="ExternalInput").ap()
    edT_d = nc.dram_tensor("edT", [DIN, TOKP], f8, kind="ExternalInput").ap()
    idx_d = nc.dram_tensor("idxf", [1, TOKP], bf, kind="ExternalInput").ap()
    cv_d = nc.dram_tensor("cvals", [P, 2], f32, kind="ExternalInput").ap()
    WT_d = nc.dram_tensor("WT", [DIN, DOUT], bf, kind="ExternalInput").ap()
    ldT_d = nc.dram_tensor("ldT", [DIN, NJ], f8, kind="ExternalInput").ap()
    lu_d = nc.dram_tensor("luB", [KJ_PAD, DOUT], bf, kind="ExternalInput").ap()
    out_d = nc.dram_tensor("out", [TOKP, DOUT], bf, kind="ExternalOutput").ap()

    with tile.TileContext(nc) as tc:
        with (
            tc.tile_pool(name="consts", bufs=1) as consts,
            tc.tile_pool(name="outsb", bufs=5) as outsb,
        ):
            # Warmup operand: no input dependency, so the dummy matmuls can
            # start immediately and pull the HAM clock gate to 2.4 GHz while
            # the DMAs stream.
            warm = consts.tile([P, 512], bf, tag="warm")
            nc.vector.memset(warm[:], 0.0)
            ones = consts.tile([1, P], bf, tag="ones")
            nc.vector.memset(ones[:], 1.0)

            # ---- input DMAs.  Two HWDGE rings (sync + scalar sequencers),
            # each FIFO: program order is priority order.  Critical path
            # first: routing ids, then the A.T chain (ld/ed), then wave-A
            # operands (lu, x/W k0..2), then wave-B (x/W k3..5).
            idx_sb = consts.tile([1, TOKP], bf, tag="idx")
            nc.sync.dma_start(idx_sb[:], idx_d[:, :])
            cvals = consts.tile([P, 2], f32, tag="cvals")
            nc.sync.dma_start(cvals[:], cv_d[:, :])
            ld_all = consts.tile([P, KD, NJ], f8, tag="ld")
            nc.sync.dma_start(ld_all[:],
                              ldT_d.rearrange("(k p) j -> p k j", p=P))
            ed_a = consts.tile([P, KA, TOKP], f8, tag="ed_a")
            nc.sync.dma_start(ed_a[:],
                              edT_d[0:KA * P, :].rearrange(
                                  "(k p) t -> p k t", p=P))
            ed_b = consts.tile([P, KD - KA, TOKP], f8, tag="ed_b")
            nc.scalar.dma_start(ed_b[:],
                                edT_d[KA * P:DIN, :].rearrange(
                                    "(k p) t -> p k t", p=P))
            lu_all = consts.tile([P, 2, DOUT], bf, tag="lu")
            nc.scalar.dma_start(lu_all[:],
                                lu_d.rearrange("(j p) o -> p j o", p=P))
            xa = consts.tile([P, KA, TOKP], bf, tag="xa")
            nc.scalar.dma_start(xa[:],
                                xT_d[0:KA * P, :].rearrange(
                                    "(k p) t -> p k t", p=P))
            Wk = []
            for k in range(KD):
                t_ = consts.tile([P, DOUT], bf, tag=f"W{k}", name=f"Wk{k}")
                Wk.append(t_)
            # W k-tiles alternate rings so early k's land first.
            nc.sync.dma_start(Wk[0][:], WT_d[0:P, :])
            nc.scalar.dma_start(Wk[1][:], WT_d[P:2 * P, :])
            nc.sync.dma_start(Wk[2][:], WT_d[2 * P:3 * P, :])
            xb = consts.tile([P, KD - KA, TOKP], bf, tag="xb")
            nc.scalar.dma_start(xb[:],
                                xT_d[KA * P:DIN, :].rearrange(
                                    "(k p) t -> p k t", p=P))
            nc.sync.dma_start(Wk[3][:], WT_d[3 * P:4 * P, :])
            nc.scalar.dma_start(Wk[4][:], WT_d[4 * P:5 * P, :])
            nc.sync.dma_start(Wk[5][:], WT_d[5 * P:6 * P, :])

            masks = [consts.tile([P, TOKP], bf, tag=f"mask{j}",
                                 name=f"mask{j}") for j in range(2)]
            MT = [consts.tile([P, TOKP], bf, tag=f"MT{j}", name=f"MT{j}")
                  for j in range(2)]
            # Chunk-1 rows 72..127 pair with luB rows 200..255: zero them,
            # then the bias ones-row at 96 (b_lin sits at luB[224]); the
            # mask-mul below overwrites rows 0..71 (lora j=128..199).
            nc.vector.memset(MT[1][64:P, :], 0.0)
            nc.vector.memset(MT[1][96:97, :], 1.0)

            # Dummy matmuls: ~3us of PE busy warms the clock gate.
            with tc.tile_pool(name="warm_ps", bufs=1, space="PSUM") as wpool:
                wps = wpool.tile([P, 512], f32, tag="wps")
                for _ in range(7):
                    nc.tensor.matmul(wps[:], warm[:, 0:P], warm[:],
                                     start=True, stop=True)

            # Broadcast token->concept ids to all 128 partitions with a
            # K=1 matmul (exact: small ints in bf16), then build the
            # one-hot masks with is_equal; masks carry 1/LD_SCALE to undo
            # the fp8 pre-scale on lora_down.
            with tc.tile_pool(name="idx_ps", bufs=2, space="PSUM") as ipool:
                for nh in range(2):
                    nsl = slice(nh * NH, (nh + 1) * NH)
                    ips = ipool.tile([P, NH], f32, tag="ips")
                    nc.tensor.matmul(ips[:], ones[:], idx_sb[:, nsl],
                                     start=True, stop=True)
                    for jc in range(2):
                        nc.vector.tensor_scalar(
                            masks[jc][:, nsl], ips[:],
                            cvals[:, jc:jc + 1], 1.0 / LD_SCALE,
                            mybir.AluOpType.is_equal, mybir.AluOpType.mult)

            # A.T[(c,r), t] = (64*lora_down_flat).T @ ed.T for all concepts,
            # masked into MT (the routed "down" activations, transposed).
            with tc.tile_pool(name="at_ps", bufs=4, space="PSUM") as at_pool:
                for jc in range(2):
                    jp = P if jc == 0 else NJ - P  # 128, 72
                    jsl = slice(jc * P, jc * P + jp)
                    for nh in range(2):
                        nsl = slice(nh * NH, (nh + 1) * NH)
                        at = at_pool.tile([P, NH], f32, tag="at")
                        for k in range(KD):
                            ed_t = (ed_a[:, k, nsl] if k < KA
                                    else ed_b[:, k - KA, nsl])
                            nc.tensor.matmul(
                                at[:jp, :], ld_all[:, k, jsl], ed_t,
                                start=(k == 0), stop=(k == KD - 1))
                        nc.vector.tensor_tensor(
                            MT[jc][:jp, nsl], at[:jp, :], masks[jc][:jp, nsl],
                            mybir.AluOpType.mult)

            # Main accumulation, two waves per (t, n) so PSUM banks recycle
            # while the late W/x k-tiles are still in flight:
            #   wave A: up0+up1 (MT/lu ready early) + org k0..2 -> copy osb
            #   wave B: org k3..5 -> DVE-add into osb -> DMA out
            # Within a wave the k/jc loop is OUTER and the n-chunk loop
            # inner, so consecutive matmuls share the stationary operand.
            osbs = []
            with tc.tile_pool(name="out_ps", bufs=8, space="PSUM") as out_pool:
                for ti in range(NT):
                    tsl = slice(ti * P, (ti + 1) * P)
                    osb = outsb.tile([P, DOUT], bf, tag="osb")
                    osbs.append(osb)
                    ps3 = [out_pool.tile([P, 512], f32, tag="ops",
                                         name=f"psA{ci}")
                           for ci in range(len(N_CHUNKS))]
                    nmm = 2 + KA
                    for i in range(nmm):
                        for ci, (n0, nw) in enumerate(N_CHUNKS):
                            if i < 2:
                                lhsT = MT[i][:, tsl]
                                rhs = lu_all[:, i, n0:n0 + nw]
                            else:
                                k = i - 2
                                lhsT = xa[:, k, tsl]
                                rhs = Wk[k][:, n0:n0 + nw]
                            nc.tensor.matmul(ps3[ci][:, :nw], lhsT, rhs,
                                             start=(i == 0),
                                             stop=(i == nmm - 1))
                    for ci, (n0, nw) in enumerate(N_CHUNKS):
                        nc.any.tensor_copy(out=osb[:, n0:n0 + nw],
                                           in_=ps3[ci][:, :nw])
                for ti in range(NT):
                    tsl = slice(ti * P, (ti + 1) * P)
                    osb = osbs[ti]
                    ps3 = [out_pool.tile([P, 512], f32, tag="ops",
                                         name=f"psB{ci}")
                           for ci in range(len(N_CHUNKS))]
                    for i, k in enumerate(range(KA, KD)):
                        for ci, (n0, nw) in enumerate(N_CHUNKS):
                            nc.tensor.matmul(
                                ps3[ci][:, :nw], xb[:, k - KA, tsl],
                                Wk[k][:, n0:n0 + nw],
                                start=(i == 0), stop=(i == KD - KA - 1))
                    for ci, (n0, nw) in enumerate(N_CHUNKS):
                        nc.any.tensor_tensor(
                            osb[:, n0:n0 + nw], ps3[ci][:, :nw],
                            osb[:, n0:n0 + nw], mybir.AluOpType.add)
                    nc.sync.dma_start(out_d[tsl, :], osb[:, :])

    nc.compile()
    return nc


def get_bass():
    if "v2" not in _cache:
        _cache["v2"] = _build_bass()
    return _cache["v2"]


def make_in_maps(x, edit_direction, concept_idx, lora_down, lora_up, W, b_lin):
    """Host-side sharding + layout prep (no reference FLOPs)."""
    bf = mybir.dt.np(mybir.dt.bfloat16)
    f8 = mybir.dt.np(mybir.dt.float8e4)
    x = np.asarray(x, dtype=np.float32)
    ed = np.asarray(edit_direction, dtype=np.float32)
    idx = np.asarray(concept_idx)
    ld = np.asarray(lora_down, dtype=np.float32)
    lup = np.asarray(lora_up, dtype=np.float32)
    W = np.asarray(W, dtype=np.float32)
    b = np.asarray(b_lin, dtype=np.float32)

    WT = np.ascontiguousarray(W.T.astype(bf))                    # [768, 1280]
    ldT = np.ascontiguousarray(
        (ld.transpose(1, 0, 2).reshape(DIN, NJ) * LD_SCALE).astype(f8))
    luB = np.zeros((KJ_PAD, DOUT), dtype=np.float32)
    luB[:NJ] = lup.reshape(NJ, DOUT) * SCALE                     # exact x0.25
    luB[128 + 96] = b                                            # bias row
    luB = luB.astype(bf)
    cv = np.full(2 * P, -1.0, dtype=np.float32)
    cv[:NJ] = np.arange(NJ, dtype=np.float32) // RANK
    cvals = np.ascontiguousarray(cv.reshape(2, P).T)             # [128, 2]

    in_maps = []
    for c in range(N_CORES):
        sl = slice(c * BPC, (c + 1) * BPC)
        xs = np.zeros((TOKP, DIN), dtype=np.float32)
        xs[:TOK] = x[sl].reshape(TOK, DIN)
        eds = np.zeros((TOKP, DIN), dtype=np.float32)
        eds[:TOK] = ed[sl].reshape(TOK, DIN)
        idxs = np.full(TOKP, IDX_PAD, dtype=np.float32)
        idxs[:TOK] = idx[sl].reshape(TOK).astype(np.float32)
        in_maps.append({
            "xT": np.ascontiguousarray(xs.T.astype(bf)),
            "edT": np.ascontiguousarray(eds.T.astype(f8)),
            "idxf": np.ascontiguousarray(idxs.reshape(1, TOKP).astype(bf)),
            "cvals": cvals,
            "WT": WT,
            "ldT": ldT,
            "luB": luB,
        })
    return in_maps


def kernel(x, edit_direction, concept_idx, lora_down, lora_up, W, b_lin,
           _trace=False, _mm_dtype=None, _lora_dtype=None):
    nc = get_bass()
    in_maps = make_in_maps(x, edit_direction, concept_idx, lora_down,
                           lora_up, W, b_lin)
    res = run_bass_kernel_spmd(nc, in_maps, core_ids=list(range(N_CORES)),
                               trace=_trace)
    out = np.concatenate(
        [r["out"][:TOK].astype(np.float32) for r in res.results], axis=0)
    out = out.reshape(B, T, DOUT)
    if _trace:
        kernel.last_results = res
    return out


# revision 7
# speedup vs baseline: 1.0742x; 1.0742x over previous
"""Trainium2 Bass kernel for CPELayer_ResAG (concept-routed LoRA edit layer).

Computation (per token t with concept c = concept_idx[t]):
    down = edit_direction[t] @ lora_down[c]          # [768]@[768,4] -> [4]
    up   = down @ lora_up[c]                         # [4]@[4,1280]  -> [1280]
    out  = x[t] @ W.T + b_lin + 0.25 * up

Strategy: data-parallel over batch across 8 cores (616 tokens/core, padded
to 640 = 5 x 128 token tiles).  The routed LoRA runs densely over all 50
concepts: A.T[(c,r), t] = lora_down_flat.T @ ed.T, masked on-device with a
one-hot built from is_equal against the broadcast token->concept ids (the
MoE routing), then contracted with lora_up_flat on the tensor engine into
the same PSUM as the org matmul.  The bias rides along as one extra
contraction row (a constant row in the masked operand, b_lin row in luB).

Perf notes (vs the fp32r v1 at 51.3us):
 - org operands and the output are bf16 (same 1 cycle/row on the PE as
   fp32r, half the HBM bytes); the whole LoRA branch (ed, lora_down x64,
   lora_up x8) is fp8e4 -- it contributes ~0.7% of the output scale, so
   fp8 rounding there is invisible.  Scales are exact powers of two,
   undone by the one-hot mask value (2^-9) and the x0.125 bias row.
 - A.T and the up-matmul run fp8 DoubleRow: 256 contraction rows per
   pass, so 2 bf16 matmuls collapse into 1.
 - every input is pre-swizzled host-side into its exact SBUF image so
   each DMA is a contiguous [128, L]-line copy (the v2 ld load generated
   768 x 200B descriptors -- far below the 512B line-rate threshold --
   and alone delayed the A.T chain by ~6us).
 - loads are split across BOTH HWDGE rings (sync + scalar sequencers),
   critical-path first; each ring drains ~183 GB/s when both are active.
 - the token->concept ids broadcast across partitions via a K=1 matmul
   (ones.T @ idx) instead of a 128-descriptor broadcast DMA.
 - dummy warmup matmuls during the load phase pull the HAM clock gate to
   2.4 GHz before the real matmuls start.

All host-side work is layout only (pad / transpose / reshape / dtype
casts); every FLOP of the reference runs on device.
"""

import sys
import types

import numpy as np

import concourse.mybir as mybir
import concourse.tile as tile
from concourse import bacc
from concourse.bass_utils import run_bass_kernel_spmd

# If BASS_TRACE is set in the environment, run_bass_kernel_spmd imports
# antenv.axon_hooks, which some containers lack; stub it (None hook ->
# tracing is skipped gracefully, execution unaffected).
try:
    import antenv.axon_hooks  # noqa: F401
except ImportError:
    _m = types.ModuleType("antenv.axon_hooks")
    _m.get_axon_ntff_profile_hook = lambda: None
    _m.set_axon_ntff_profile_hook = lambda h: None
    sys.modules["antenv.axon_hooks"] = _m

# Problem shapes (hardcoded per spec nn_CPELayer_ResAG_19335942766951)
N_CORES = 8
B, T, DIN, DOUT = 64, 77, 768, 1280
N_CONCEPTS, RANK = 50, 4
SCALE = 0.25                # alpha/rank = 1/4, exact power of two
BPC = B // N_CORES          # batches per core = 8
TOK = BPC * T               # real tokens per core = 616
TOKP = 640                  # padded tokens = 5 full 128-token tiles
NT = TOKP // 128            # 5 token tiles
NJ = N_CONCEPTS * RANK      # 200 flattened (concept, rank) rows
NJP = 208                   # j padded so the DoubleRow plane stride is %16
P = 128
KD = DIN // P               # 6 k-tiles of the d_in contraction
KA = 3                      # org k-tiles in wave A (k0..2), wave B k3..5
NH = TOKP // 2              # 320-token halves for the A.T psum tiles
LD_SCALE = 64.0             # fp8 pre-scale on lora_down
LU_SCALE = 8.0              # fp8 pre-scale on 0.25*lora_up (and bias)
MASK_VAL = 1.0 / (LD_SCALE * LU_SCALE)   # 2^-9, exact in bf16
BIAS_VAL = 1.0 / LU_SCALE                # 0.125, exact in fp8
N_CHUNKS = [(0, 512), (512, 512), (1024, 256)]
IDX_PAD = -2.0              # pad-token id; cvals pad is -1 so never equal
N_WARM = 4
DR = mybir.MatmulPerfMode.DoubleRow

_cache = {}


def _build_bass():
    nc = bacc.Bacc("TRN2", target_bir_lowering=False, debug=False,
                   num_devices=N_CORES)
    f32 = mybir.dt.float32
    bf = mybir.dt.bfloat16
    f8 = mybir.dt.float8e4

    # All DRAM inputs are pre-swizzled host-side to the SBUF image:
    # partition dim first, contiguous fat lines.
    x_a_d = nc.dram_tensor("x_a", [P, KA, TOKP], bf, kind="ExternalInput").ap()
    x_b_d = nc.dram_tensor("x_b", [P, KD - KA, TOKP], bf,
                           kind="ExternalInput").ap()
    W_a_d = nc.dram_tensor("W_a", [P, KA, DOUT], bf, kind="ExternalInput").ap()
    W_b_d = nc.dram_tensor("W_b", [P, KD - KA, DOUT], bf,
                           kind="ExternalInput").ap()
    ed_a_d = nc.dram_tensor("ed_a", [P, 2, TOKP], f8,
                            kind="ExternalInput").ap()
    ed_b_d = nc.dram_tensor("ed_b", [P, 4, TOKP], f8,
                            kind="ExternalInput").ap()
    ld_d = nc.dram_tensor("ld", [P, KD, NJP], f8, kind="ExternalInput").ap()
    lu_d = nc.dram_tensor("luB", [P, 2, DOUT], f8, kind="ExternalInput").ap()
    idx_d = nc.dram_tensor("idxf", [1, TOKP], bf, kind="ExternalInput").ap()
    cv_d = nc.dram_tensor("cvals", [P, 2], f32, kind="ExternalInput").ap()
    out_d = nc.dram_tensor("out", [TOKP, DOUT], bf, kind="ExternalOutput").ap()

    with tile.TileContext(nc) as tc:
        with (
            tc.tile_pool(name="consts", bufs=1) as consts,
            tc.tile_pool(name="outsb", bufs=5) as outsb,
        ):
            # Warmup operand: no input dependency, so the dummy matmuls
            # start immediately.
            warm = consts.tile([P, 512], bf, tag="warm")
            nc.vector.memset(warm[:], 0.0)
            ones = consts.tile([1, P], bf, tag="ones")
            nc.vector.memset(ones[:], 1.0)

            # ---- input DMAs.  Two HWDGE rings, each FIFO in program
            # order.  sync: idx, cv, ld, ed_a, Wa, xb ; scalar: ed_b, lu,
            # xa, Wb.  (A.T needs ld+ed first; wave A then lu/xa/Wa.)
            idx_sb = consts.tile([1, TOKP], bf, tag="idx")
            nc.sync.dma_start(idx_sb[:], idx_d[:, :])
            cvals = consts.tile([P, 2], f32, tag="cvals")
            nc.sync.dma_start(cvals[:], cv_d[:, :])
            ld_all = consts.tile([P, KD, NJP], f8, tag="ld")
            nc.sync.dma_start(ld_all[:], ld_d[:, :, :])
            ed_a = consts.tile([P, 2, TOKP], f8, tag="ed_a")
            nc.sync.dma_start(ed_a[:], ed_a_d[:, :, :])
            ed_b = consts.tile([P, 4, TOKP], f8, tag="ed_b")
            nc.scalar.dma_start(ed_b[:], ed_b_d[:, :, :])
            lu_all = consts.tile([P, 2, DOUT], f8, tag="lu")
            nc.scalar.dma_start(lu_all[:], lu_d[:, :, :])
            xa = consts.tile([P, KA, TOKP], bf, tag="xa")
            nc.scalar.dma_start(xa[:], x_a_d[:, :, :])
            Wa = consts.tile([P, KA, DOUT], bf, tag="Wa")
            nc.sync.dma_start(Wa[:], W_a_d[:, :, :])
            xb = consts.tile([P, KD - KA, TOKP], bf, tag="xb")
            nc.sync.dma_start(xb[:], x_b_d[:, :, :])
            Wb = consts.tile([P, KD - KA, DOUT], bf, tag="Wb")
            nc.scalar.dma_start(Wb[:], W_b_d[:, :, :])

            masks = [consts.tile([P, TOKP], bf, tag=f"mask{j}",
                                 name=f"mask{j}") for j in range(2)]
            # MT holds both 128-row j-planes side by side for DoubleRow.
            MT = consts.tile([P, 2, TOKP], f8, tag="MT")
            # Plane-1 rows 72..127 pair with luB rows 200..255: zero them,
            # then the bias row at 96 carries 1/LU_SCALE (b_lin*LU_SCALE
            # sits at luB[224]); the mask-mul overwrites rows 0..71.
            nc.vector.memset(MT[64:P, 1, :], 0.0)
            nc.vector.memset(MT[96:97, 1, :], BIAS_VAL)

            with tc.tile_pool(name="warm_ps", bufs=1, space="PSUM") as wpool:
                wps = wpool.tile([P, 512], f32, tag="wps")
                for _ in range(N_WARM):
                    nc.tensor.matmul(wps[:], warm[:, 0:P], warm[:],
                                     start=True, stop=True)

            # Broadcast token->concept ids to all 128 partitions with a
            # K=1 matmul (exact: small ints in bf16), then build the
            # one-hot masks; mask value MASK_VAL undoes the fp8 scales.
            with tc.tile_pool(name="idx_ps", bufs=2, space="PSUM") as ipool:
                for nh in range(2):
                    nsl = slice(nh * NH, (nh + 1) * NH)
                    ips = ipool.tile([P, NH], f32, tag="ips")
                    nc.tensor.matmul(ips[:], ones[:], idx_sb[:, nsl],
                                     start=True, stop=True)
                    for jc in range(2):
                        nc.vector.tensor_scalar(
                            masks[jc][:, nsl], ips[:],
                            cvals[:, jc:jc + 1], MASK_VAL,
                            mybir.AluOpType.is_equal, mybir.AluOpType.mult)

            # A.T = (64*lora_down_flat).T @ ed.T for all concepts, three
            # DoubleRow passes of 256 contraction rows, masked into MT.
            ed_pair = [ed_a[:, 0:2, :], ed_b[:, 0:2, :], ed_b[:, 2:4, :]]
            with tc.tile_pool(name="at_ps", bufs=4, space="PSUM") as at_pool:
                for jc in range(2):
                    jp = P if jc == 0 else NJ - P  # 128, 72
                    jsl = slice(jc * P, jc * P + jp)
                    for nh in range(2):
                        nsl = slice(nh * NH, (nh + 1) * NH)
                        at = at_pool.tile([P, NH], f32, tag="at")
                        for kk in range(KD // 2):
                            nc.tensor.matmul(
                                at[:jp, :],
                                ld_all[:, 2 * kk:2 * kk + 2, jsl],
                                ed_pair[kk][:, :, nsl],
                                start=(kk == 0), stop=(kk == KD // 2 - 1),
                                perf_mode=DR)
                        nc.vector.tensor_tensor(
                            MT[:jp, jc, nsl], at[:jp, :], masks[jc][:jp, nsl],
                            mybir.AluOpType.mult)

            # Main accumulation, two waves per (t, n) so PSUM banks recycle
            # while the late W/x k-tiles are still in flight:
            #   wave A: up (one DoubleRow matmul) + org k0..2 -> copy osb
            #   wave B: org k3..5 -> DVE-add into osb -> DMA out
            # The stationary loop is outer, n-chunks inner, so consecutive
            # matmuls share the stationary operand.
            osbs = []
            with tc.tile_pool(name="out_ps", bufs=8, space="PSUM") as out_pool:
                for ti in range(NT):
                    tsl = slice(ti * P, (ti + 1) * P)
                    osb = outsb.tile([P, DOUT], bf, tag="osb")
                    osbs.append(osb)
                    ps3 = [out_pool.tile([P, 512], f32, tag="ops",
                                         name=f"psA{ci}")
                           for ci in range(len(N_CHUNKS))]
                    nmm = 1 + KA
                    for i in range(nmm):
                        for ci, (n0, nw) in enumerate(N_CHUNKS):
                            if i == 0:
                                nc.tensor.matmul(
                                    ps3[ci][:, :nw], MT[:, :, tsl],
                                    lu_all[:, :, n0:n0 + nw],
                                    start=True, stop=False, perf_mode=DR)
                            else:
                                k = i - 1
                                nc.tensor.matmul(
                                    ps3[ci][:, :nw], xa[:, k, tsl],
                                    Wa[:, k, n0:n0 + nw],
                                    start=False, stop=(i == nmm - 1))
                    for ci, (n0, nw) in enumerate(N_CHUNKS):
                        nc.any.tensor_copy(out=osb[:, n0:n0 + nw],
                                           in_=ps3[ci][:, :nw])
                for ti in range(NT):
                    tsl = slice(ti * P, (ti + 1) * P)
                    osb = osbs[ti]
                    ps3 = [out_pool.tile([P, 512], f32, tag="ops",
                                         name=f"psB{ci}")
                           for ci in range(len(N_CHUNKS))]
                    for i, k in enumerate(range(KA, KD)):
                        for ci, (n0, nw) in enumerate(N_CHUNKS):
                            nc.tensor.matmul(
                                ps3[ci][:, :nw], xb[:, k - KA, tsl],
                                Wb[:, k - KA, n0:n0 + nw],
                                start=(i == 0), stop=(i == KD - KA - 1))
                    for ci, (n0, nw) in enumerate(N_CHUNKS):
                        nc.any.tensor_tensor(
                            osb[:, n0:n0 + nw], ps3[ci][:, :nw],
                            osb[:, n0:n0 + nw], mybir.AluOpType.add)
                    nc.sync.dma_start(out_d[tsl, :], osb[:, :])

    nc.compile()
    return nc


def get_bass():
    if "v3" not in _cache:
        _cache["v3"] = _build_bass()
    return _cache["v3"]


def _swizzle_kp(a, k0, k1, np_dt):
    """[DIN, L] -> SBUF image [P, k1-k0, L] for k-tiles k0..k1."""
    return np.ascontiguousarray(
        a.reshape(KD, P, -1)[k0:k1].transpose(1, 0, 2).astype(np_dt))


def make_in_maps(x, edit_direction, concept_idx, lora_down, lora_up, W, b_lin):
    """Host-side sharding + layout prep (no reference FLOPs)."""
    bf = mybir.dt.np(mybir.dt.bfloat16)
    f8 = mybir.dt.np(mybir.dt.float8e4)
    x = np.asarray(x, dtype=np.float32)
    ed = np.asarray(edit_direction, dtype=np.float32)
    idx = np.asarray(concept_idx)
    ld = np.asarray(lora_down, dtype=np.float32)
    lup = np.asarray(lora_up, dtype=np.float32)
    W = np.asarray(W, dtype=np.float32)
    b = np.asarray(b_lin, dtype=np.float32)

    WT = W.T                                                     # [768, 1280]
    W_a = _swizzle_kp(WT, 0, KA, bf)
    W_b = _swizzle_kp(WT, KA, KD, bf)
    ldT = np.zeros((DIN, NJP), dtype=np.float32)
    ldT[:, :NJ] = ld.transpose(1, 0, 2).reshape(DIN, NJ) * LD_SCALE
    ld_sw = _swizzle_kp(ldT, 0, KD, f8)
    luB = np.zeros((2 * P, DOUT), dtype=np.float32)
    luB[:NJ] = lup.reshape(NJ, DOUT) * (SCALE * LU_SCALE)
    luB[128 + 96] = b * LU_SCALE                                 # bias row
    luB = np.ascontiguousarray(
        luB.reshape(2, P, DOUT).transpose(1, 0, 2).astype(f8))   # [P, 2, DOUT]
    cv = np.full(2 * P, -1.0, dtype=np.float32)
    cv[:NJ] = np.arange(NJ, dtype=np.float32) // RANK
    cvals = np.ascontiguousarray(cv.reshape(2, P).T)             # [128, 2]

    in_maps = []
    for c in range(N_CORES):
        sl = slice(c * BPC, (c + 1) * BPC)
        xs = np.zeros((TOKP, DIN), dtype=np.float32)
        xs[:TOK] = x[sl].reshape(TOK, DIN)
        xT = xs.T                                                # [768, 640]
        eds = np.zeros((TOKP, DIN), dtype=np.float32)
        eds[:TOK] = ed[sl].reshape(TOK, DIN)
        edT = eds.T
        idxs = np.full(TOKP, IDX_PAD, dtype=np.float32)
        idxs[:TOK] = idx[sl].reshape(TOK).astype(np.float32)
        in_maps.append({
            "x_a": _swizzle_kp(xT, 0, KA, bf),
            "x_b": _swizzle_kp(xT, KA, KD, bf),
            "W_a": W_a,
            "W_b": W_b,
            "ed_a": _swizzle_kp(edT, 0, 2, f8),
            "ed_b": _swizzle_kp(edT, 2, KD, f8),
            "ld": ld_sw,
            "luB": luB,
            "idxf": np.ascontiguousarray(idxs.reshape(1, TOKP).astype(bf)),
            "cvals": cvals,
        })
    return in_maps


def kernel(x, edit_direction, concept_idx, lora_down, lora_up, W, b_lin,
           _trace=False, _mm_dtype=None, _lora_dtype=None):
    nc = get_bass()
    in_maps = make_in_maps(x, edit_direction, concept_idx, lora_down,
                           lora_up, W, b_lin)
    res = run_bass_kernel_spmd(nc, in_maps, core_ids=list(range(N_CORES)),
                               trace=_trace)
    out = np.concatenate(
        [r["out"][:TOK].astype(np.float32) for r in res.results], axis=0)
    out = out.reshape(B, T, DOUT)
    if _trace:
        kernel.last_results = res
    return out


# revision 8
# speedup vs baseline: 1.2108x; 1.1272x over previous
"""Trainium2 Bass kernel for CPELayer_ResAG (concept-routed LoRA edit layer).

Computation (per token t with concept c = concept_idx[t]):
    down = edit_direction[t] @ lora_down[c]          # [768]@[768,4] -> [4]
    up   = down @ lora_up[c]                         # [4]@[4,1280]  -> [1280]
    out  = x[t] @ W.T + b_lin + 0.25 * up

Strategy: data-parallel over batch across 8 cores (616 tokens/core, padded
to 640 = 5 x 128 token tiles).  The routed LoRA runs densely over all 50
concepts: A.T[(c,r), t] = lora_down_flat.T @ ed.T, masked on-device with a
one-hot built from is_equal against the broadcast token->concept ids (the
MoE routing), then contracted with lora_up_flat on the tensor engine into
the same PSUM as the org matmul.  The bias rides along as one extra
contraction row (a constant row in the masked operand, b_lin row in luB).

Perf notes (vs the fp32r v1 at 51.3us):
 - org operands and the output are bf16 (same 1 cycle/row on the PE as
   fp32r, half the HBM bytes); the whole LoRA branch (ed, lora_down x64,
   lora_up x8) is fp8e4 -- it contributes ~0.7% of the output scale, so
   fp8 rounding there is invisible.  Scales are exact powers of two,
   undone by the one-hot mask value (2^-9) and the x0.125 bias row.
 - A.T and the up-matmul run fp8 DoubleRow: 256 contraction rows/pass.
 - DMA traffic is packed host-side into a few [128, L] blobs with fat
   (2.5-11.5KB) contiguous partition lines: SDMA descriptor overhead
   (~150-240ns each) makes thin-line transfers run at a fraction of the
   ~360 GB/s HBM rate (v2's 200B-line lora_down load alone cost ~6us,
   a [128, 2] fp32 load ~2us).
 - loads split across BOTH HWDGE rings (sync + scalar), critical-path
   first: the A.T operands land first, then lu/x/W for wave A.
 - token->concept ids and the concept row-values broadcast/transpose via
   K=1 matmuls from one single-line DMA instead of per-partition loads.
 - dummy warmup matmuls during the load phase pull the HAM clock gate to
   2.4 GHz before the real matmuls start.

All host-side work is layout only (pad / transpose / reshape / dtype
casts); every FLOP of the reference runs on device.
"""

import sys
import types

import numpy as np

import concourse.mybir as mybir
import concourse.tile as tile
from concourse import bacc
from concourse.bass_utils import run_bass_kernel_spmd

# If BASS_TRACE is set in the environment, run_bass_kernel_spmd imports
# antenv.axon_hooks, which some containers lack; stub it (None hook ->
# tracing is skipped gracefully, execution unaffected).
try:
    import antenv.axon_hooks  # noqa: F401
except ImportError:
    _m = types.ModuleType("antenv.axon_hooks")
    _m.get_axon_ntff_profile_hook = lambda: None
    _m.set_axon_ntff_profile_hook = lambda h: None
    sys.modules["antenv.axon_hooks"] = _m

# Problem shapes (hardcoded per spec nn_CPELayer_ResAG_19335942766951)
N_CORES = 8
B, T, DIN, DOUT = 64, 77, 768, 1280
N_CONCEPTS, RANK = 50, 4
SCALE = 0.25                # alpha/rank = 1/4, exact power of two
BPC = B // N_CORES          # batches per core = 8
TOK = BPC * T               # real tokens per core = 616
TOKP = 640                  # padded tokens = 5 full 128-token tiles
NT = TOKP // 128            # 5 token tiles
NJ = N_CONCEPTS * RANK      # 200 flattened (concept, rank) rows
NJP = 208                   # j padded so the DoubleRow plane stride is %16
P = 128
KD = DIN // P               # 6 k-tiles of the d_in contraction
KA = 3                      # org k-tiles in wave A (k0..2), wave B k3..5
NH = TOKP // 2              # 320-token halves for the A.T psum tiles
LD_SCALE = 64.0             # fp8 pre-scale on lora_down
LU_SCALE = 8.0              # fp8 pre-scale on 0.25*lora_up (and bias)
MASK_VAL = 1.0 / (LD_SCALE * LU_SCALE)   # 2^-9, exact in bf16
BIAS_VAL = 1.0 / LU_SCALE                # 0.125, exact in fp8
N_CHUNKS = [(0, 512), (512, 512), (1024, 256)]
IDX_PAD = -2.0              # pad-token id; cvals pad is -1 so never equal
N_WARM = 4
DR = mybir.MatmulPerfMode.DoubleRow

# fp8 pack layout (per-partition offsets, elements == bytes)
LD_LEN = KD * NJP           # 1248
ED_LEN = TOKP               # 640 per k-tile
L8A = LD_LEN + 2 * ED_LEN   # pack a: ld + ed k0..1   (2528 B lines)
L8B = 4 * ED_LEN            # pack b: ed k2..5        (2560 B lines)
# bf16 pack for wave B (elements)
PBX = (KD - KA) * TOKP      # 1920 xb elements
PBW = (KD - KA) * DOUT      # 3840 Wb elements
IDXCV = TOKP + 2 * P        # idx row + cv row, single line

_cache = {}


def _build_bass():
    nc = bacc.Bacc("TRN2", target_bir_lowering=False, debug=False,
                   num_devices=N_CORES)
    f32 = mybir.dt.float32
    bf = mybir.dt.bfloat16
    f8 = mybir.dt.float8e4

    # All DRAM inputs are pre-swizzled host-side to the SBUF image:
    # partition dim first, contiguous fat lines.
    l8a_d = nc.dram_tensor("l8a", [P, L8A], f8, kind="ExternalInput").ap()
    l8b_d = nc.dram_tensor("l8b", [P, L8B], f8, kind="ExternalInput").ap()
    lu_d = nc.dram_tensor("luB", [P, 2 * DOUT], f8, kind="ExternalInput").ap()
    xa_d = nc.dram_tensor("xa", [P, KA * TOKP], bf, kind="ExternalInput").ap()
    Wa_d = nc.dram_tensor("Wa", [P, KA * DOUT], bf, kind="ExternalInput").ap()
    pb_d = nc.dram_tensor("pb", [P, PBX + PBW], bf, kind="ExternalInput").ap()
    idxcv_d = nc.dram_tensor("idxcv", [1, IDXCV], bf,
                             kind="ExternalInput").ap()
    out_d = nc.dram_tensor("out", [TOKP, DOUT], bf, kind="ExternalOutput").ap()

    with tile.TileContext(nc) as tc:
        with (
            tc.tile_pool(name="consts", bufs=1) as consts,
            tc.tile_pool(name="outsb", bufs=5) as outsb,
        ):
            # ---- input DMAs first in program order (= issue priority).
            # Two HWDGE rings, each FIFO.  sync: idxcv, l8a, l8b, Wa, pb;
            # scalar: lu, xa, then the output stores.
            idxcv = consts.tile([1, IDXCV], bf, tag="idxcv")
            nc.sync.dma_start(idxcv[:], idxcv_d[:, :])
            l8 = consts.tile([P, L8A + L8B], f8, tag="l8")
            nc.sync.dma_start(l8[:, 0:L8A], l8a_d[:, :])
            lu_fl = consts.tile([P, 2 * DOUT], f8, tag="lu")
            nc.scalar.dma_start(lu_fl[:], lu_d[:, :])
            nc.sync.dma_start(l8[:, L8A:L8A + L8B], l8b_d[:, :])
            xa_fl = consts.tile([P, KA * TOKP], bf, tag="xa")
            nc.scalar.dma_start(xa_fl[:], xa_d[:, :])
            Wa_fl = consts.tile([P, KA * DOUT], bf, tag="Wa")
            nc.sync.dma_start(Wa_fl[:], Wa_d[:, :])
            pb = consts.tile([P, PBX + PBW], bf, tag="pb")
            nc.sync.dma_start(pb[:], pb_d[:, :])

            # typed views into the packs
            ld3 = l8[:, 0:LD_LEN].rearrange("p (k j) -> p k j", k=KD)
            ed01 = l8[:, LD_LEN:L8A].rearrange("p (k t) -> p k t", k=2)
            ed25 = l8[:, L8A:L8A + L8B].rearrange("p (k t) -> p k t", k=4)
            ed_pair = [ed01[:, 0:2, :], ed25[:, 0:2, :], ed25[:, 2:4, :]]
            lu_all = lu_fl.rearrange("p (j o) -> p j o", j=2)
            xa = xa_fl.rearrange("p (k t) -> p k t", k=KA)
            Wa = Wa_fl.rearrange("p (k o) -> p k o", k=KA)
            xb = pb[:, 0:PBX].rearrange("p (k t) -> p k t", k=KD - KA)
            Wb = pb[:, PBX:PBX + PBW].rearrange("p (k o) -> p k o", k=KD - KA)

            warm = consts.tile([P, 512], bf, tag="warm")
            nc.vector.memset(warm[:], 0.0)
            ones = consts.tile([1, P], bf, tag="ones")
            nc.vector.memset(ones[:], 1.0)
            ones1 = consts.tile([1, 1], bf, tag="ones1")
            nc.vector.memset(ones1[:], 1.0)
            cvals = consts.tile([P, 2], f32, tag="cvals")

            masks = [consts.tile([P, TOKP], bf, tag=f"mask{j}",
                                 name=f"mask{j}") for j in range(2)]
            # MT holds both 128-row j-planes side by side for DoubleRow.
            MT = consts.tile([P, 2, TOKP], f8, tag="MT")
            # Plane-1 rows 72..127 pair with luB rows 200..255: zero them,
            # then the bias row at 96 carries 1/LU_SCALE (b_lin*LU_SCALE
            # sits at luB[224]); the mask-mul overwrites rows 0..71.
            nc.vector.memset(MT[64:P, 1, :], 0.0)
            nc.vector.memset(MT[96:97, 1, :], BIAS_VAL)

            with tc.tile_pool(name="warm_ps", bufs=1, space="PSUM") as wpool:
                wps = wpool.tile([P, 512], f32, tag="wps")
                for _ in range(N_WARM):
                    nc.tensor.matmul(wps[:], warm[:, 0:P], warm[:],
                                     start=True, stop=True)

            # Transpose the concept row-values (one K=1 matmul per 128-row
            # half) and broadcast the token ids (K=1, ones stationary);
            # then build the one-hot masks.  MASK_VAL undoes the fp8
            # scales on lora_down/lora_up.
            with tc.tile_pool(name="idx_ps", bufs=4, space="PSUM") as ipool:
                for jc in range(2):
                    cvp = ipool.tile([P, 1], f32, tag="cvp")
                    nc.tensor.matmul(
                        cvp[:], idxcv[:, TOKP + jc * P:TOKP + (jc + 1) * P],
                        ones1[:], start=True, stop=True)
                    nc.any.tensor_copy(out=cvals[:, jc:jc + 1], in_=cvp[:])
                for nh in range(2):
                    nsl = slice(nh * NH, (nh + 1) * NH)
                    ips = ipool.tile([P, NH], f32, tag="ips")
                    nc.tensor.matmul(ips[:], ones[:], idxcv[:, nsl],
                                     start=True, stop=True)
                    for jc in range(2):
                        nc.vector.tensor_scalar(
                            masks[jc][:, nsl], ips[:],
                            cvals[:, jc:jc + 1], MASK_VAL,
                            mybir.AluOpType.is_equal, mybir.AluOpType.mult)

            # A.T = (64*lora_down_flat).T @ ed.T for all concepts, three
            # DoubleRow passes of 256 contraction rows, masked into MT.
            # kk outer with 4 open psum groups, so the kk=0 pass runs as
            # soon as the first pack lands while ed k2..5 is in flight.
            with tc.tile_pool(name="at_ps", bufs=4, space="PSUM") as at_pool:
                ats = {}
                for jc in range(2):
                    jp = P if jc == 0 else NJ - P  # 128, 72
                    for nh in range(2):
                        ats[jc, nh] = at_pool.tile([P, NH], f32, tag="at",
                                                   name=f"at{jc}{nh}")
                for kk in range(KD // 2):
                    for jc in range(2):
                        jp = P if jc == 0 else NJ - P
                        jsl = slice(jc * P, jc * P + jp)
                        for nh in range(2):
                            nsl = slice(nh * NH, (nh + 1) * NH)
                            nc.tensor.matmul(
                                ats[jc, nh][:jp, :],
                                ld3[:, 2 * kk:2 * kk + 2, jsl],
                                ed_pair[kk][:, :, nsl],
                                start=(kk == 0), stop=(kk == KD // 2 - 1),
                                perf_mode=DR)
                for jc in range(2):
                    jp = P if jc == 0 else NJ - P
                    for nh in range(2):
                        nsl = slice(nh * NH, (nh + 1) * NH)
                        nc.vector.tensor_tensor(
                            MT[:jp, jc, nsl], ats[jc, nh][:jp, :],
                            masks[jc][:jp, nsl], mybir.AluOpType.mult)

            # Main accumulation, two waves per (t, n) so PSUM banks recycle
            # while the late W/x k-tiles are still in flight:
            #   wave A: up (one DoubleRow matmul) + org k0..2 -> copy osb
            #   wave B: org k3..5 -> DVE-add into osb -> DMA out
            # The stationary loop is outer, n-chunks inner, so consecutive
            # matmuls share the stationary operand.
            osbs = []
            with tc.tile_pool(name="out_ps", bufs=8, space="PSUM") as out_pool:
                for ti in range(NT):
                    tsl = slice(ti * P, (ti + 1) * P)
                    osb = outsb.tile([P, DOUT], bf, tag="osb")
                    osbs.append(osb)
                    ps3 = [out_pool.tile([P, 512], f32, tag="ops",
                                         name=f"psA{ci}")
                           for ci in range(len(N_CHUNKS))]
                    nmm = 1 + KA
                    for i in range(nmm):
                        for ci, (n0, nw) in enumerate(N_CHUNKS):
                            if i == 0:
                                nc.tensor.matmul(
                                    ps3[ci][:, :nw], MT[:, :, tsl],
                                    lu_all[:, :, n0:n0 + nw],
                                    start=True, stop=False, perf_mode=DR)
                            else:
                                k = i - 1
                                nc.tensor.matmul(
                                    ps3[ci][:, :nw], xa[:, k, tsl],
                                    Wa[:, k, n0:n0 + nw],
                                    start=False, stop=(i == nmm - 1))
                    for ci, (n0, nw) in enumerate(N_CHUNKS):
                        nc.any.tensor_copy(out=osb[:, n0:n0 + nw],
                                           in_=ps3[ci][:, :nw])
                for ti in range(NT):
                    tsl = slice(ti * P, (ti + 1) * P)
                    osb = osbs[ti]
                    ps3 = [out_pool.tile([P, 512], f32, tag="ops",
                                         name=f"psB{ci}")
                           for ci in range(len(N_CHUNKS))]
                    for i, k in enumerate(range(KA, KD)):
                        for ci, (n0, nw) in enumerate(N_CHUNKS):
                            nc.tensor.matmul(
                                ps3[ci][:, :nw], xb[:, k - KA, tsl],
                                Wb[:, k - KA, n0:n0 + nw],
                                start=(i == 0), stop=(i == KD - KA - 1))
                    for ci, (n0, nw) in enumerate(N_CHUNKS):
                        nc.any.tensor_tensor(
                            osb[:, n0:n0 + nw], ps3[ci][:, :nw],
                            osb[:, n0:n0 + nw], mybir.AluOpType.add)
                    nc.scalar.dma_start(out_d[tsl, :], osb[:, :])

    nc.compile()
    return nc


def get_bass():
    if "v4" not in _cache:
        _cache["v4"] = _build_bass()
    return _cache["v4"]


def _swz(a, k0, k1):
    """[DIN, L] -> [P, (k1-k0)*L] SBUF-image lines for k-tiles k0..k1."""
    return a.reshape(KD, P, -1)[k0:k1].transpose(1, 0, 2).reshape(P, -1)


def make_in_maps(x, edit_direction, concept_idx, lora_down, lora_up, W, b_lin):
    """Host-side sharding + layout prep (no reference FLOPs)."""
    bf = mybir.dt.np(mybir.dt.bfloat16)
    f8 = mybir.dt.np(mybir.dt.float8e4)
    x = np.asarray(x, dtype=np.float32)
    ed = np.asarray(edit_direction, dtype=np.float32)
    idx = np.asarray(concept_idx)
    ld = np.asarray(lora_down, dtype=np.float32)
    lup = np.asarray(lora_up, dtype=np.float32)
    W = np.asarray(W, dtype=np.float32)
    b = np.asarray(b_lin, dtype=np.float32)

    WT = W.T                                                     # [768, 1280]
    Wa = np.ascontiguousarray(_swz(WT, 0, KA).astype(bf))
    Wb = _swz(WT, KA, KD)
    ldT = np.zeros((DIN, NJP), dtype=np.float32)
    ldT[:, :NJ] = ld.transpose(1, 0, 2).reshape(DIN, NJ) * LD_SCALE
    ld_sw = _swz(ldT, 0, KD)                                     # [P, 1248]
    luB = np.zeros((2 * P, DOUT), dtype=np.float32)
    luB[:NJ] = lup.reshape(NJ, DOUT) * (SCALE * LU_SCALE)
    luB[128 + 96] = b * LU_SCALE                                 # bias row
    luB = np.ascontiguousarray(
        luB.reshape(2, P, DOUT).transpose(1, 0, 2).reshape(P, -1).astype(f8))
    cvrow = np.full(2 * P, -1.0, dtype=np.float32)
    cvrow[:NJ] = np.arange(NJ, dtype=np.float32) // RANK

    in_maps = []
    for c in range(N_CORES):
        sl = slice(c * BPC, (c + 1) * BPC)
        xs = np.zeros((TOKP, DIN), dtype=np.float32)
        xs[:TOK] = x[sl].reshape(TOK, DIN)
        xT = xs.T                                                # [768, 640]
        eds = np.zeros((TOKP, DIN), dtype=np.float32)
        eds[:TOK] = ed[sl].reshape(TOK, DIN)
        edT = eds.T
        ed_sw = _swz(edT, 0, KD)                                 # [P, 3840]
        idxcv = np.full(IDXCV, IDX_PAD, dtype=np.float32)
        idxcv[:TOK] = idx[sl].reshape(TOK).astype(np.float32)
        idxcv[TOKP:] = cvrow
        l8a = np.concatenate([ld_sw, ed_sw[:, :2 * ED_LEN]], axis=1)
        pbm = np.concatenate([_swz(xT, KA, KD), Wb], axis=1)
        in_maps.append({
            "l8a": np.ascontiguousarray(l8a.astype(f8)),
            "l8b": np.ascontiguousarray(ed_sw[:, 2 * ED_LEN:].astype(f8)),
            "luB": luB,
            "xa": np.ascontiguousarray(_swz(xT, 0, KA).astype(bf)),
            "Wa": Wa,
            "pb": np.ascontiguousarray(pbm.astype(bf)),
            "idxcv": np.ascontiguousarray(
                idxcv.reshape(1, IDXCV).astype(bf)),
        })
    return in_maps


def kernel(x, edit_direction, concept_idx, lora_down, lora_up, W, b_lin,
           _trace=False, _mm_dtype=None, _lora_dtype=None):
    nc = get_bass()
    in_maps = make_in_maps(x, edit_direction, concept_idx, lora_down,
                           lora_up, W, b_lin)
    res = run_bass_kernel_spmd(nc, in_maps, core_ids=list(range(N_CORES)),
                               trace=_trace)
    out = np.concatenate(
        [r["out"][:TOK].astype(np.float32) for r in res.results], axis=0)
    out = out.reshape(B, T, DOUT)
    if _trace:
        kernel.last_results = res
    return out
